# revision 37
# baseline (speedup 1.0000x reference)
"""Trainium2 Bass kernel for windowed attention with decomposed relative
position bias (ViTDet-style), batch-parallel across 8 NeuronCores.

Reference computation (per batch b):
    qkv = x @ qkv_w.T + qkv_b ; split into q, k, v heads (12 heads, hd=64)
    attn = (q * hd**-0.5) @ k.T + rel_h bias + rel_w bias
    out  = softmax(attn) @ v ; out @ proj_w.T + proj_b

Design (per core = one batch element, bf16 datapath, f32 PSUM):
  - QR [128, 12*1024]: rows 0:64 qT per head, 64:96 rel_hT, 96:128 rel_wT.
    KE likewise holds kT plus 0/1 indicator rows E, so the decomposed
    rel-pos bias is fused into the S.T matmul as extra contraction rows.
  - ACT is the pacing engine: 96 exps of [128,1024] ~= 107us.  Schedule
    goal: first exp as early as possible, ACT exp-pure during the main
    phase, all other eviction work on DVE (+ACT only in startup/tail).
  - DMAs are batched ([128, ct, ...] host layouts) so the single sync
    HWDGE FIFO is not a startup serializer.
  - rel tables: computed for heads 0-5 in the startup (gates head_st(0))
    and heads 6-11 during the early main phase (DVE has slack there);
    rel_w evictions are strided (4B/64B lines) and cost ~4x contiguous.
  - softmax skips max-subtraction (logits are small by construction);
    denominator rides as a ones-column appended to v (65-wide U.T
    stationary); normalization = DVE copy + reciprocal_approx_fast +
    gpsimd partition-broadcast + multiply fused into the U.T eviction.
  - U.T chains run nt-outer/ch-inner so each exp tile is consumed as
    soon as ACT produces it (the last head's U.T finishes ~one MM pair
    after the last exp), and LDWEIGHTS is shared per nt.
  - Tail: proj chains are emitted interleaved with the last head's U.T
    so the PE stays dense/warm through the end.
  - PSUM: 4 banks double-buffer the S.T tiles, 4 banks shared by all
    other matmul chains (qkv/rel/v/U.T); proj reuses the st slots.
"""

import numpy as np

NH, HD, C, HW = 12, 64, 768, 1024
H = W = 32
NCORES = 8
CT = C // 128          # 6 contraction tiles
VW = NH * 65           # 780: v block width per n-tile (64 cols + ones col)

_CACHE = {}


def _build(loop_k=0):
    import concourse.bass as bass
    import concourse.mybir as mybir
    import concourse.tile as tile
    from concourse import bacc

    f32 = mybir.dt.float32
    bf16 = mybir.dt.bfloat16
    EXP = mybir.ActivationFunctionType.Exp

    nc = bacc.Bacc(num_devices=NCORES)
    # all host-side layouts are [128 partitions, ct-chunked free]
    d_xT = nc.dram_tensor("xT", [128, CT * HW], bf16, kind="ExternalInput")
    d_wqk = nc.dram_tensor("wqk", [128, CT * 2 * C], bf16, kind="ExternalInput")
    d_wv = nc.dram_tensor("wv", [128, CT * C], bf16, kind="ExternalInput")
    d_wp = nc.dram_tensor("wp", [128, CT * C], bf16, kind="ExternalInput")
    d_rha = nc.dram_tensor("rha", [HD, HW], bf16, kind="ExternalInput")
    d_rwa = nc.dram_tensor("rwa", [HD, HW], bf16, kind="ExternalInput")
    d_ep = nc.dram_tensor("ep", [HD, NH * HW], bf16, kind="ExternalInput")
    d_out = nc.dram_tensor("out", [HW, C], f32, kind="ExternalOutput")

    def body(tc):
        with (
            tc.tile_pool(name="persist", bufs=1) as pp,
            tc.tile_pool(name="sb", bufs=2) as sb,
            tc.tile_pool(name="expp", bufs=24) as ep34,
            tc.tile_pool(name="ps", bufs=2, space="PSUM") as ps,
        ):
            QR = pp.tile([128, NH * HW], bf16, tag="QR")
            KE = pp.tile([128, NH * HW], bf16, tag="KE")
            VSB = pp.tile([128, 8, VW], bf16, tag="VSB")
            OUTT = pp.tile([128, 6, HW], bf16, tag="OUTT")

            wqk3 = d_wqk.ap().rearrange("p (ct c) -> p ct c", ct=CT)
            wv3 = d_wv.ap().rearrange("p (ct c) -> p ct c", ct=CT)
            wp3 = d_wp.ap().rearrange("p (ct c) -> p ct c", ct=CT)
            xT3 = d_xT.ap().rearrange("p (ct n) -> p ct n", ct=CT)

            def stream_w(half, jtp):
                """One [128, CT, 256] weight tile (256 output features of
                the q/k projection), one batched DMA."""
                t = sb.tile([128, CT, 256], bf16, tag="wqk", bufs=4)
                c0 = half * C + jtp * 256
                nc.sync.dma_start(out=t, in_=wqk3[:, :, c0:c0 + 256])
                return t

            # ---- initial DMAs (one sync HWDGE FIFO, priority order)
            wq0 = stream_w(0, 0)
            XT = pp.tile([128, CT, HW], bf16, tag="XT")
            nc.sync.dma_start(out=XT[:, 0, :], in_=xT3[:, 0, :])
            nc.sync.dma_start(out=XT[:, 1:CT, :], in_=xT3[:, 1:CT, :])
            wq1 = stream_w(0, 1)
            wq2 = stream_w(0, 2)
            rha = pp.tile([HD, HW], bf16, tag="rha")
            rwa = pp.tile([HD, HW], bf16, tag="rwa")
            nc.sync.dma_start(out=rha, in_=d_rha.ap())
            nc.sync.dma_start(out=rwa, in_=d_rwa.ap())
            nc.sync.dma_start(
                out=KE[64:128, 0:4 * HW], in_=d_ep.ap()[:, 0:4 * HW])
            k0w = stream_w(1, 0)
            # preload the exp table set on ACT now (otherwise the ~2.7us
            # ACT_TABLE_LOAD sits right before the first real exp)
            warm = sb.tile([1, 2], f32, tag="warm")
            nc.vector.memset(warm, 0.0)
            nc.scalar.activation(warm, warm[:], EXP)

            # ones columns of VSB (col 64 of each 65-wide head block)
            ones_ap = VSB[:].rearrange("p n (h c) -> p n h c", c=65)[:, :, :, 64:65]
            nc.vector.memset(ones_ap, 1.0)

            def qk_jtp(half, jtp, wsl, a_only=None):
                """One pair of 256 output features of the q (half=0) or k
                (half=1) projection.  Pre-attention blocks borrow the wide
                st PSUM slots; blocks that run while attention is live
                keep the narrow mm slots."""
                dest = (QR, KE)[half]
                for a in ((0, 1) if a_only is None else (a_only,)):
                    wide = True  # all q/k blocks run pre-attention now
                    hA = (jtp * 2 + a) * 2
                    if wide:
                        pw = ps.tile([128, 1024], f32, tag="st",
                                     name=f"qk{half}_{jtp}_{a}")
                        pss = [pw[:, 0:512], pw[:, 512:1024]]
                    else:
                        pss = [ps.tile([128, 512], f32, tag="mm", bufs=4,
                                       name=f"qk{half}_{jtp}_{a}_{c}")
                               for c in range(2)]
                    for ct in range(CT):
                        for ch in range(2):
                            nc.tensor.matmul(
                                pss[ch],
                                wsl[:, ct, a * 128:(a + 1) * 128],
                                XT[:, ct, ch * 512:(ch + 1) * 512],
                                start=(ct == 0), stop=(ct == CT - 1),
                            )
                    if wide:
                        nc.vector.tensor_copy(
                            dest[0:64, hA * HW:hA * HW + HW], pw[0:64, :])
                        nc.scalar.copy(
                            dest[0:64, (hA + 1) * HW:(hA + 1) * HW + HW],
                            pw[64:128, :])
                        continue
                    for ch in range(2):
                        p = pss[ch]
                        m0 = ch * 512
                        nc.vector.tensor_copy(
                            dest[0:64, hA * HW + m0:hA * HW + m0 + 512], p[0:64, :])
                        odd = dest[0:64, (hA + 1) * HW + m0:(hA + 1) * HW + m0 + 512]
                        nc.vector.tensor_copy(odd, p[64:128, :])

            # ---- rel tables (packed 4x via column tiling), head range
            # [h0, h1); evictions on ACT+DVE in startup, all-DVE in main
            q3 = QR[0:64, :].rearrange("p (j a b) -> p j a b", j=NH, b=32)
            d3h = QR[64:96, :].rearrange("p (j a b) -> p j a b", j=NH, b=32)
            d3w = QR[96:128, :].rearrange("p (j a b) -> p j a b", j=NH, b=32)

            def rel_group(g, h0, h1, act_share):
                nh = h1 - h0
                prh = ps.tile([128, 512], f32, tag="mm", bufs=4,
                              name=f"relh{g}_{h0}")[:, 0:nh * 32]
                prw = ps.tile([128, 512], f32, tag="mm", bufs=4,
                              name=f"relw{g}_{h0}")[:, 0:nh * 32]
                for j in range(4):
                    r = 4 * g + j
                    nc.tensor.matmul(
                        prh[32 * j:32 * (j + 1), :],
                        rha[:, r * 32:(r + 1) * 32], q3[:, h0:h1, r, :],
                        start=True, stop=True, tile_position=(0, 32 * j))
                    nc.tensor.matmul(
                        prw[32 * j:32 * (j + 1), :],
                        rwa[:, r * 32:(r + 1) * 32], q3[:, h0:h1, :, r],
                        start=True, stop=True, tile_position=(0, 32 * j))
                for j in range(4):
                    r = 4 * g + j
                    # evictions split across ACT/DVE (startup: both idle)
                    if j < 2:
                        nc.scalar.copy(d3h[:, h0:h1, r, :],
                                       prh[32 * j:32 * (j + 1), :])
                        nc.vector.tensor_copy(d3w[:, h0:h1, :, r],
                                              prw[32 * j:32 * (j + 1), :])
                    else:
                        nc.vector.tensor_copy(d3h[:, h0:h1, r, :],
                                              prh[32 * j:32 * (j + 1), :])
                        nc.scalar.copy(d3w[:, h0:h1, :, r],
                                       prw[32 * j:32 * (j + 1), :])

            def v_stream(c2):
                t = sb.tile([128, CT, 384], bf16, tag="wv", bufs=2)
                nc.sync.dma_start(
                    out=t, in_=wv3[:, :, c2 * 384:(c2 + 1) * 384])
                return t

            def v_mg(c2, wsl, mg):
                pss = [ps.tile([128, 384], f32, tag="mm", bufs=4,
                               name=f"v{c2}_{mg}_{i}") for i in range(2)]
                for ct in range(CT):
                    for i in range(2):
                        mt = mg * 2 + i
                        nc.tensor.matmul(
                            pss[i], XT[:, ct, mt * 128:(mt + 1) * 128],
                            wsl[:, ct, :],
                            start=(ct == 0), stop=(ct == CT - 1))
                for i in range(2):
                    mt = mg * 2 + i
                    dst = VSB[:, mt, :].rearrange("p (h c) -> p h c", c=65)
                    nc.vector.tensor_copy(
                        dst[:, 6 * c2:6 * c2 + 6, 0:64],
                        pss[i][:].rearrange("p (h c) -> p h c", c=64))

            # ---- attention, software-pipelined: S.T+exp vs U.T+normalize
            exs = {}

            def head_st(h):
                ex = []
                for nt in range(8):
                    st = ps.tile([128, 1024], f32, tag="st", name=f"st{h}_{nt}")
                    for ch in range(2):
                        nc.tensor.matmul(
                            st[:, ch * 512:(ch + 1) * 512],
                            KE[:, h * HW + nt * 128:h * HW + (nt + 1) * 128],
                            QR[:, h * HW + ch * 512:h * HW + (ch + 1) * 512],
                            start=True, stop=True)
                    e = ep34.tile([128, 1024], bf16, tag="expT", bufs=24)
                    nc.scalar.activation(e, st, EXP)
                    ex.append(e)
                exs[h] = ex

            def ut_mms(h, nts):
                """U.T accumulation chain MMs for head h over nt in nts;
                nt-outer/ch-inner shares LDWEIGHTS per nt."""
                ex, uts = exs[h]
                for nt in nts:
                    for ch in range(2):
                        nc.tensor.matmul(
                            uts[ch], VSB[:, nt, h * 65:(h + 1) * 65],
                            ex[nt][:, ch * 512:(ch + 1) * 512],
                            start=(nt == 0), stop=(nt == 7))

            def ut_norm(h):
                _, uts = exs.pop(h)
                r0 = (h % 2) * 64
                for ch in range(2):
                    ut = uts[ch]
                    dsb = sb.tile([1, 512], f32, tag="dsb")
                    nc.vector.tensor_copy(dsb, ut[64:65, :])
                    rsb = sb.tile([1, 512], f32, tag="rsb")
                    nc.vector.reciprocal_approx_fast(rsb, dsb[:])
                    rb = sb.tile([64, 512], f32, tag="rb")
                    nc.gpsimd.partition_broadcast(rb, rsb[:])
                    nc.vector.tensor_mul(
                        OUTT[r0:r0 + 64, h // 2, ch * 512:(ch + 1) * 512],
                        ut[0:64, :], rb[:])

            def head_ut(h):
                ex = exs[h]
                uts = [ps.tile([65, 512], f32, tag="mm", bufs=4,
                               name=f"ut{h}_{ch}") for ch in range(2)]
                exs[h] = (ex, uts)
                ut_mms(h, range(8))
                ut_norm(h)

            # ================= schedule =================
            # Front-loaded: ALL projections (q, k, v) and rel tables run
            # in the startup where the PE is otherwise eviction-wall
            # idle; the main phase is pure attention (st+ut = ~88us PE
            # < 107us ACT exp floor -> ACT-paced, no slot crunches).
            qk_jtp(0, 0, wq0)
            qk_jtp(0, 1, wq1)
            qk_jtp(0, 2, wq2)
            wv0 = v_stream(0)
            nc.sync.dma_start(
                out=KE[64:128, 4 * HW:12 * HW], in_=d_ep.ap()[:, 4 * HW:12 * HW])
            k1w = stream_w(1, 1)
            qk_jtp(1, 0, k0w)                 # k heads 0-3
            wv1 = v_stream(1)
            k2w = stream_w(1, 2)
            WP = pp.tile([128, CT, C], bf16, tag="WP")
            nc.sync.dma_start(out=WP, in_=wp3[:])
            # all k chains back-to-back: their st-slot evictions only
            # compete with each other (interleaving them with rel groups
            # stalled the slot recycle behind rel's eviction bursts)
            qk_jtp(1, 1, k1w)                 # k heads 4-7
            qk_jtp(1, 2, k2w)                 # k heads 8-11
            # rel groups (mm slots) interleaved with v blocks (mm slots);
            # their MMs fill the PE during the rel eviction wall
            for g in range(8):
                rel_group(g, 0, 12, act_share=True)
                if g < 4:
                    v_mg(0, wv0, g)
                else:
                    v_mg(1, wv1, g - 4)
            # ---- pure-attention main phase
            head_st(0)
            head_st(1)
            head_ut(0)
            head_st(2)
            head_ut(1)
            head_st(3)
            head_ut(2)
            head_st(4)
            head_ut(3)
            head_st(5)
            head_ut(4)
            head_st(6)
            head_ut(5)
            head_st(7)
            head_ut(6)
            head_st(8)
            head_ut(7)
            head_st(9)
            head_ut(8)
            head_st(10)
            head_ut(9)
            head_st(11)
            head_ut(10)
            # ---- tail: 4-deep proj pipeline (2 st slots + 4 mm slots),
            # interleaved with the last head's U.T so the PE stays dense
            # through the final exps and HAM stays warm
            ex11 = exs[11]
            uts11 = [ps.tile([65, 512], f32, tag="mm", bufs=4,
                             name=f"ut11_{ch}") for ch in range(2)]
            exs[11] = (ex11, uts11)
            ut_mms(11, range(6))
            pjs = {}

            def pj_alloc(mt, kind):
                if kind == "st":
                    pf = ps.tile([128, 1024], f32, tag="st", name=f"pj{mt}")
                    pjs[mt] = (pf[:, 0:512], pf[:, 512:768], pf)
                else:
                    a = ps.tile([128, 512], f32, tag="mm", bufs=4,
                                name=f"pj{mt}a")
                    b = ps.tile([128, 512], f32, tag="mm", bufs=4,
                                name=f"pj{mt}b")
                    pjs[mt] = (a[:], b[:, 0:256], None)

            def pj_chain(mt, jts):
                pa, pb, _ = pjs[mt]
                for pf, o3, n3 in ((pa, 0, 512), (pb, 512, 256)):
                    for jt in jts:
                        nc.tensor.matmul(
                            pf, OUTT[:, jt, mt * 128:(mt + 1) * 128],
                            WP[:, jt, o3:o3 + n3],
                            start=(jt == 0), stop=(jt == CT - 1))

            def pj_evict(mt):
                pa, pb, _ = pjs.pop(mt)
                f = sb.tile([128, C], f32, tag="ftile")
                nc.scalar.copy(f[:, 0:512], pa)
                nc.scalar.copy(f[:, 512:768], pb)
                nc.sync.dma_start(
                    out=d_out.ap()[mt * 128:(mt + 1) * 128, :], in_=f)

            pj_alloc(0, "st")
            pj_chain(0, range(5))
            pj_alloc(1, "st")
            pj_chain(1, range(5))
            ut_mms(11, range(6, 8))
            ut_norm(11)
            pj_alloc(2, "mm")
            pj_chain(2, range(5))
            pj_alloc(3, "mm")
            pj_chain(3, range(5))
            for mt in range(4):
                pj_chain(mt, range(5, CT))
                pj_evict(mt)
            for mt in range(4, 8):
                pj_alloc(mt, "st" if mt % 2 == 0 else "mm")
                pj_chain(mt, range(CT))
                pj_evict(mt)

    with tile.TileContext(nc) as tc:
        if loop_k and loop_k > 1:
            with tc.For_i(0, loop_k, 1):
                body(tc)
        else:
            body(tc)

    nc.compile()
    return nc


def _prep(x, qkv_w, qkv_b, proj_w, proj_b, rel_pos_h, rel_pos_w):
    import ml_dtypes
    f = lambda a: np.asarray(a, dtype=np.float32)
    x, qkv_w, proj_w = f(x), f(qkv_w), f(proj_w)
    rel_pos_h, rel_pos_w = f(rel_pos_h), f(rel_pos_w)
    assert not np.any(f(qkv_b)) and not np.any(f(proj_b)), \
        "nonzero qkv/proj bias not supported by this kernel build"

    bf = ml_dtypes.bfloat16
    B = x.shape[0]

    def ctfold(a):
        # [C, N] -> [128, CT*N] with a[ct*128+p, n] at [p, ct, n]
        N = a.shape[1]
        return np.ascontiguousarray(
            a.reshape(CT, 128, N).transpose(1, 0, 2).reshape(128, CT * N))

    xT = np.ascontiguousarray(
        x.reshape(B, HW, C).transpose(0, 2, 1))           # (B, C, HW)
    xT = np.stack([ctfold(xT[b]) for b in range(B)]).astype(bf)
    wqk = ctfold(np.concatenate(
        [qkv_w[0:C] * np.float32(HD ** -0.5), qkv_w[C:2 * C]], 0).T).astype(bf)
    wv = ctfold(qkv_w[2 * C:3 * C].T).astype(bf)
    wp = ctfold(proj_w.T).astype(bf)

    idx = np.arange(32)[:, None] - np.arange(32)[None, :] + 31   # (h, k)
    sc = np.float32(HD ** 0.5)
    rha = np.ascontiguousarray(
        (rel_pos_h[idx] * sc).transpose(2, 0, 1).reshape(HD, HW)).astype(bf)
    rwa = np.ascontiguousarray(
        (rel_pos_w[idx] * sc).transpose(2, 0, 1).reshape(HD, HW)).astype(bf)

    # E patterns: rows 0..31 block indicator (n//32 == r), rows 32..63 stripe
    # indicator (n%32 == r); these turn the precomputed rel_hT/rel_wT rows of
    # the S.T rhs into the broadcast bias layout during the fused matmul.
    ep = np.zeros((HD, HW), bf)
    n = np.arange(HW)
    ep[n // 32, n] = 1.0
    ep[32 + n % 32, n] = 1.0
    ep = np.ascontiguousarray(np.tile(ep, (1, NH)))
    return xT, {"wqk": wqk, "wv": wv, "wp": wp, "rha": rha, "rwa": rwa, "ep": ep}


def _make_runner(nc):
    """Build a cached jitted 8-core runner for the compiled Bass module."""
    import jax
    import concourse.mybir as mybir
    from concourse.bass2jax import (
        _bass_exec_p, install_neuronx_cc_hook, partition_id_tensor)
    from jax.experimental.shard_map import shard_map
    from jax.sharding import Mesh, PartitionSpec

    install_neuronx_cc_hook()
    partition_name = nc.partition_id_tensor.name if nc.partition_id_tensor else None
    in_names, out_names, out_avals, zero_outs = [], [], [], []
    for alloc in nc.m.functions[0].allocations:
        if not isinstance(alloc, mybir.MemoryLocationSet):
            continue
        name = alloc.memorylocations[0].name
        if alloc.kind == "ExternalInput":
            if name != partition_name:
                in_names.append(name)
        elif alloc.kind == "ExternalOutput":
            shape = tuple(alloc.tensor_shape)
            dtype = mybir.dt.np(alloc.dtype)
            out_names.append(name)
            out_avals.append(jax.core.ShapedArray(shape, dtype))
            zero_outs.append(np.zeros(shape, dtype))
    n_params = len(in_names)
    n_outs = len(out_avals)
    all_in_names = list(in_names) + list(out_names)
    if partition_name is not None:
        all_in_names.append(partition_name)

    def _body(*args):
        operands = list(args)
        if partition_name is not None:
            operands.append(partition_id_tensor())
        return tuple(_bass_exec_p.bind(
            *operands,
            out_avals=tuple(out_avals),
            in_names=tuple(all_in_names),
            out_names=tuple(out_names),
            lowering_input_output_aliases=(),
            sim_require_finite=True,
            sim_require_nnan=True,
            nc=nc,
        ))

    devices = jax.devices()[:NCORES]
    mesh = Mesh(np.asarray(devices), ("core",))
    spec = jax.sharding.NamedSharding(mesh, PartitionSpec("core"))
    sharded = jax.jit(
        shard_map(_body, mesh=mesh,
                  in_specs=(PartitionSpec("core"),) * (n_params + n_outs),
                  out_specs=(PartitionSpec("core"),) * n_outs,
                  check_rep=False),
        keep_unused=True)

    class Runner:
        def __init__(self):
            self._dev_args = None

        def put(self, in_maps):
            concat_in = [
                np.concatenate([np.asarray(m[name]) for m in in_maps], axis=0)
                for name in in_names
            ]
            concat_zeros = [
                np.zeros((NCORES * z.shape[0], *z.shape[1:]), z.dtype)
                for z in zero_outs
            ]
            self._dev_args = [jax.device_put(a, spec)
                              for a in concat_in + concat_zeros]
            jax.block_until_ready(self._dev_args)

        def exec(self):
            out = sharded(*self._dev_args)
            jax.block_until_ready(out)
            return out

        def run(self, in_maps):
            self.put(in_maps)
            out_arrs = [np.asarray(a) for a in self.exec()]
            self._dev_args = None
            return [
                {name: out_arrs[i].reshape(NCORES, *out_avals[i].shape)[c]
                 for i, name in enumerate(out_names)}
                for c in range(NCORES)
            ]

    return Runner()


def get_runner(_loop_k=0):
    key = ("runner", _loop_k)
    if key not in _CACHE:
        nc = _build(loop_k=_loop_k)
        _CACHE[key] = _make_runner(nc)
    return _CACHE[key]


def kernel(x, qkv_w, qkv_b, proj_w, proj_b, rel_pos_h, rel_pos_w, _loop_k=0):
    xT, shared = _prep(x, qkv_w, qkv_b, proj_w, proj_b, rel_pos_h, rel_pos_w)
    B = xT.shape[0]
    assert B == NCORES
    runner = get_runner(_loop_k)
    results = runner.run([{"xT": xT[b], **shared} for b in range(B)])
    out = np.stack([results[b]["out"] for b in range(B)], 0)
    return out.reshape(B, H, W, C)


# revision 38
# speedup vs baseline: 1.0483x; 1.0483x over previous
"""Trainium2 Bass kernel for windowed attention with decomposed relative
position bias (ViTDet-style), batch-parallel across 8 NeuronCores.

Reference computation (per batch b):
    qkv = x @ qkv_w.T + qkv_b ; split into q, k, v heads (12 heads, hd=64)
    attn = (q * hd**-0.5) @ k.T + rel_h bias + rel_w bias
    out  = softmax(attn) @ v ; out @ proj_w.T + proj_b

Design (per core = one batch element, bf16 datapath, f32 PSUM):
  - QR [128, 12*1024]: rows 0:64 qT per head, 64:96 rel_hT, 96:128 rel_wT.
    KE likewise holds kT plus 0/1 indicator rows E, so the decomposed
    rel-pos bias is fused into the S.T matmul as extra contraction rows.
  - ACT is the pacing engine: 96 exps of [128,1024] ~= 107us.  Schedule
    goal: first exp as early as possible, ACT exp-pure during the main
    phase, all other eviction work on DVE (+ACT only in startup/tail).
  - DMAs are batched ([128, ct, ...] host layouts) so the single sync
    HWDGE FIFO is not a startup serializer.
  - rel tables: computed for heads 0-5 in the startup (gates head_st(0))
    and heads 6-11 during the early main phase (DVE has slack there);
    rel_w evictions are strided (4B/64B lines) and cost ~4x contiguous.
  - softmax skips max-subtraction (logits are small by construction);
    denominator rides as a ones-column appended to v (65-wide U.T
    stationary); normalization = DVE copy + reciprocal_approx_fast +
    gpsimd partition-broadcast + multiply fused into the U.T eviction.
  - U.T chains run nt-outer/ch-inner so each exp tile is consumed as
    soon as ACT produces it (the last head's U.T finishes ~one MM pair
    after the last exp), and LDWEIGHTS is shared per nt.
  - Tail: proj chains are emitted interleaved with the last head's U.T
    so the PE stays dense/warm through the end.
  - PSUM: 4 banks double-buffer the S.T tiles, 4 banks shared by all
    other matmul chains (qkv/rel/v/U.T); proj reuses the st slots.
"""

import numpy as np

NH, HD, C, HW = 12, 64, 768, 1024
H = W = 32
NCORES = 8
CT = C // 128          # 6 contraction tiles
VW = NH * 65           # 780: v block width per n-tile (64 cols + ones col)

_CACHE = {}


def _build(loop_k=0):
    import concourse.bass as bass
    import concourse.mybir as mybir
    import concourse.tile as tile
    from concourse import bacc

    f32 = mybir.dt.float32
    bf16 = mybir.dt.bfloat16
    EXP = mybir.ActivationFunctionType.Exp

    nc = bacc.Bacc(num_devices=NCORES)
    # all host-side layouts are [128 partitions, ct-chunked free]
    d_xT = nc.dram_tensor("xT", [128, CT * HW], bf16, kind="ExternalInput")
    d_wqk = nc.dram_tensor("wqk", [128, CT * 2 * C], bf16, kind="ExternalInput")
    d_wv = nc.dram_tensor("wv", [128, CT * C], bf16, kind="ExternalInput")
    d_wp = nc.dram_tensor("wp", [128, CT * C], bf16, kind="ExternalInput")
    d_rha = nc.dram_tensor("rha", [HD, HW], bf16, kind="ExternalInput")
    d_rwa = nc.dram_tensor("rwa", [HD, HW], bf16, kind="ExternalInput")
    d_ep = nc.dram_tensor("ep", [HD, NH * HW], bf16, kind="ExternalInput")
    d_out = nc.dram_tensor("out", [HW, C], f32, kind="ExternalOutput")

    def body(tc):
        with (
            tc.tile_pool(name="persist", bufs=1) as pp,
            tc.tile_pool(name="sb", bufs=2) as sb,
            tc.tile_pool(name="expp", bufs=24) as ep34,
            tc.tile_pool(name="ps", bufs=2, space="PSUM") as ps,
        ):
            QR = pp.tile([128, NH * HW], bf16, tag="QR")
            KE = pp.tile([128, NH * HW], bf16, tag="KE")
            VSB = pp.tile([128, 8, VW], bf16, tag="VSB")
            OUTT = pp.tile([128, 6, HW], bf16, tag="OUTT")

            wqk3 = d_wqk.ap().rearrange("p (ct c) -> p ct c", ct=CT)
            wv3 = d_wv.ap().rearrange("p (ct c) -> p ct c", ct=CT)
            wp3 = d_wp.ap().rearrange("p (ct c) -> p ct c", ct=CT)
            xT3 = d_xT.ap().rearrange("p (ct n) -> p ct n", ct=CT)

            def stream_w(half, jtp):
                """One [128, CT, 256] weight tile (256 output features of
                the q/k projection), one batched DMA."""
                t = sb.tile([128, CT, 256], bf16, tag="wqk", bufs=4)
                c0 = half * C + jtp * 256
                nc.sync.dma_start(out=t, in_=wqk3[:, :, c0:c0 + 256])
                return t

            # ---- initial DMAs (one sync HWDGE FIFO, priority order)
            wq0 = stream_w(0, 0)
            XT = pp.tile([128, CT, HW], bf16, tag="XT")
            nc.sync.dma_start(out=XT[:, 0, :], in_=xT3[:, 0, :])
            nc.sync.dma_start(out=XT[:, 1:CT, :], in_=xT3[:, 1:CT, :])
            wq1 = stream_w(0, 1)
            wq2 = stream_w(0, 2)
            rha = pp.tile([HD, HW], bf16, tag="rha")
            rwa = pp.tile([HD, HW], bf16, tag="rwa")
            nc.sync.dma_start(out=rha, in_=d_rha.ap())
            nc.sync.dma_start(out=rwa, in_=d_rwa.ap())
            nc.sync.dma_start(
                out=KE[64:128, 0:4 * HW], in_=d_ep.ap()[:, 0:4 * HW])
            k0w = stream_w(1, 0)
            # preload the exp table set on ACT now (otherwise the ~2.7us
            # ACT_TABLE_LOAD sits right before the first real exp)
            warm = sb.tile([1, 2], f32, tag="warm")
            nc.vector.memset(warm, 0.0)
            nc.scalar.activation(warm, warm[:], EXP)

            # ones columns of VSB (col 64 of each 65-wide head block)
            ones_ap = VSB[:].rearrange("p n (h c) -> p n h c", c=65)[:, :, :, 64:65]
            nc.vector.memset(ones_ap, 1.0)

            def qk_jtp(half, jtp, wsl, a_only=None):
                """One pair of 256 output features of the q (half=0) or k
                (half=1) projection.  Pre-attention blocks borrow the wide
                st PSUM slots; blocks that run while attention is live
                keep the narrow mm slots."""
                dest = (QR, KE)[half]
                for a in ((0, 1) if a_only is None else (a_only,)):
                    wide = True  # all q/k blocks run pre-attention now
                    hA = (jtp * 2 + a) * 2
                    if wide:
                        pw = ps.tile([128, 1024], f32, tag="st",
                                     name=f"qk{half}_{jtp}_{a}")
                        pss = [pw[:, 0:512], pw[:, 512:1024]]
                    else:
                        pss = [ps.tile([128, 512], f32, tag="mm", bufs=4,
                                       name=f"qk{half}_{jtp}_{a}_{c}")
                               for c in range(2)]
                    for ct in range(CT):
                        for ch in range(2):
                            nc.tensor.matmul(
                                pss[ch],
                                wsl[:, ct, a * 128:(a + 1) * 128],
                                XT[:, ct, ch * 512:(ch + 1) * 512],
                                start=(ct == 0), stop=(ct == CT - 1),
                            )
                    if wide:
                        nc.vector.tensor_copy(
                            dest[0:64, hA * HW:hA * HW + HW], pw[0:64, :])
                        nc.scalar.copy(
                            dest[0:64, (hA + 1) * HW:(hA + 1) * HW + HW],
                            pw[64:128, :])
                        continue
                    for ch in range(2):
                        p = pss[ch]
                        m0 = ch * 512
                        nc.vector.tensor_copy(
                            dest[0:64, hA * HW + m0:hA * HW + m0 + 512], p[0:64, :])
                        odd = dest[0:64, (hA + 1) * HW + m0:(hA + 1) * HW + m0 + 512]
                        nc.vector.tensor_copy(odd, p[64:128, :])

            # ---- rel tables (packed 4x via column tiling), head range
            # [h0, h1); evictions on ACT+DVE in startup, all-DVE in main
            q3 = QR[0:64, :].rearrange("p (j a b) -> p j a b", j=NH, b=32)
            d3h = QR[64:96, :].rearrange("p (j a b) -> p j a b", j=NH, b=32)
            d3w = QR[96:128, :].rearrange("p (j a b) -> p j a b", j=NH, b=32)

            def rel_group(g, h0, h1, act_share):
                nh = h1 - h0
                prh = ps.tile([128, 512], f32, tag="mm", bufs=4,
                              name=f"relh{g}_{h0}")[:, 0:nh * 32]
                prw = ps.tile([128, 512], f32, tag="mm", bufs=4,
                              name=f"relw{g}_{h0}")[:, 0:nh * 32]
                for j in range(4):
                    r = 4 * g + j
                    nc.tensor.matmul(
                        prh[32 * j:32 * (j + 1), :],
                        rha[:, r * 32:(r + 1) * 32], q3[:, h0:h1, r, :],
                        start=True, stop=True, tile_position=(0, 32 * j))
                    nc.tensor.matmul(
                        prw[32 * j:32 * (j + 1), :],
                        rwa[:, r * 32:(r + 1) * 32], q3[:, h0:h1, :, r],
                        start=True, stop=True, tile_position=(0, 32 * j))
                for j in range(4):
                    r = 4 * g + j
                    # evictions split across ACT/DVE (startup: both idle)
                    if j < 2:
                        nc.scalar.copy(d3h[:, h0:h1, r, :],
                                       prh[32 * j:32 * (j + 1), :])
                        nc.vector.tensor_copy(d3w[:, h0:h1, :, r],
                                              prw[32 * j:32 * (j + 1), :])
                    else:
                        nc.vector.tensor_copy(d3h[:, h0:h1, r, :],
                                              prh[32 * j:32 * (j + 1), :])
                        nc.scalar.copy(d3w[:, h0:h1, :, r],
                                       prw[32 * j:32 * (j + 1), :])

            def v_stream(c2):
                t = sb.tile([128, CT, 384], bf16, tag="wv", bufs=2)
                nc.sync.dma_start(
                    out=t, in_=wv3[:, :, c2 * 384:(c2 + 1) * 384])
                return t

            def v_mg(c2, wsl, mg):
                pss = [ps.tile([128, 384], f32, tag="mm", bufs=4,
                               name=f"v{c2}_{mg}_{i}") for i in range(2)]
                for ct in range(CT):
                    for i in range(2):
                        mt = mg * 2 + i
                        nc.tensor.matmul(
                            pss[i], XT[:, ct, mt * 128:(mt + 1) * 128],
                            wsl[:, ct, :],
                            start=(ct == 0), stop=(ct == CT - 1))
                for i in range(2):
                    mt = mg * 2 + i
                    dst = VSB[:, mt, :].rearrange("p (h c) -> p h c", c=65)
                    nc.vector.tensor_copy(
                        dst[:, 6 * c2:6 * c2 + 6, 0:64],
                        pss[i][:].rearrange("p (h c) -> p h c", c=64))

            # ---- attention, software-pipelined: S.T+exp vs U.T+normalize
            exs = {}

            def head_st(h):
                ex = []
                for nt in range(8):
                    st = ps.tile([128, 1024], f32, tag="st", name=f"st{h}_{nt}")
                    for ch in range(2):
                        nc.tensor.matmul(
                            st[:, ch * 512:(ch + 1) * 512],
                            KE[:, h * HW + nt * 128:h * HW + (nt + 1) * 128],
                            QR[:, h * HW + ch * 512:h * HW + (ch + 1) * 512],
                            start=True, stop=True)
                    e = ep34.tile([128, 1024], bf16, tag="expT", bufs=24)
                    nc.scalar.activation(e, st, EXP)
                    ex.append(e)
                exs[h] = ex

            def ut_mms(h, nts):
                """U.T accumulation chain MMs for head h over nt in nts;
                nt-outer/ch-inner shares LDWEIGHTS per nt."""
                ex, uts = exs[h]
                for nt in nts:
                    for ch in range(2):
                        nc.tensor.matmul(
                            uts[ch], VSB[:, nt, h * 65:(h + 1) * 65],
                            ex[nt][:, ch * 512:(ch + 1) * 512],
                            start=(nt == 0), stop=(nt == 7))

            def ut_norm(h):
                _, uts = exs.pop(h)
                r0 = (h % 2) * 64
                for ch in range(2):
                    ut = uts[ch]
                    dsb = sb.tile([1, 512], f32, tag="dsb")
                    nc.vector.tensor_copy(dsb, ut[64:65, :])
                    rsb = sb.tile([1, 512], f32, tag="rsb")
                    nc.vector.reciprocal_approx_fast(rsb, dsb[:])
                    rb = sb.tile([64, 512], f32, tag="rb")
                    nc.gpsimd.partition_broadcast(rb, rsb[:])
                    nc.vector.tensor_mul(
                        OUTT[r0:r0 + 64, h // 2, ch * 512:(ch + 1) * 512],
                        ut[0:64, :], rb[:])

            def head_ut(h):
                ex = exs[h]
                uts = [ps.tile([65, 512], f32, tag="mm", bufs=4,
                               name=f"ut{h}_{ch}") for ch in range(2)]
                exs[h] = (ex, uts)
                ut_mms(h, range(8))
                ut_norm(h)

            # ================= schedule =================
            # Front-loaded: ALL projections (q, k, v) and rel tables run
            # in the startup where the PE is otherwise eviction-wall
            # idle; the main phase is pure attention (st+ut = ~88us PE
            # < 107us ACT exp floor -> ACT-paced, no slot crunches).
            qk_jtp(0, 0, wq0)
            qk_jtp(0, 1, wq1)
            qk_jtp(0, 2, wq2)
            wv0 = v_stream(0)
            nc.sync.dma_start(
                out=KE[64:128, 4 * HW:12 * HW], in_=d_ep.ap()[:, 4 * HW:12 * HW])
            k1w = stream_w(1, 1)
            qk_jtp(1, 0, k0w)                 # k heads 0-3
            wv1 = v_stream(1)
            k2w = stream_w(1, 2)
            WP = pp.tile([128, CT, C], bf16, tag="WP")
            nc.sync.dma_start(out=WP, in_=wp3[:])
            # rel groups (mm slots) interleaved with v blocks (mm slots)
            # and the remaining k chains (st slots)
            for g in range(8):
                rel_group(g, 0, 12, act_share=True)
                if g < 4:
                    v_mg(0, wv0, g)
                else:
                    v_mg(1, wv1, g - 4)
                if g == 1:
                    qk_jtp(1, 1, k1w, a_only=0)   # k heads 4-5
                elif g == 3:
                    qk_jtp(1, 1, k1w, a_only=1)   # k heads 6-7
                elif g == 5:
                    qk_jtp(1, 2, k2w, a_only=0)   # k heads 8-9
                elif g == 7:
                    qk_jtp(1, 2, k2w, a_only=1)   # k heads 10-11
            # ---- pure-attention main phase
            head_st(0)
            head_st(1)
            head_ut(0)
            head_st(2)
            head_ut(1)
            head_st(3)
            head_ut(2)
            head_st(4)
            head_ut(3)
            head_st(5)
            head_ut(4)
            head_st(6)
            head_ut(5)
            head_st(7)
            head_ut(6)
            head_st(8)
            head_ut(7)
            head_st(9)
            head_ut(8)
            head_st(10)
            head_ut(9)
            head_st(11)
            head_ut(10)
            # ---- tail: 4-deep proj pipeline (2 st slots + 4 mm slots),
            # interleaved with the last head's U.T so the PE stays dense
            # through the final exps and HAM stays warm
            ex11 = exs[11]
            uts11 = [ps.tile([65, 512], f32, tag="mm", bufs=4,
                             name=f"ut11_{ch}") for ch in range(2)]
            exs[11] = (ex11, uts11)
            ut_mms(11, range(6))
            pjs = {}

            def pj_alloc(mt, kind):
                if kind == "st":
                    pf = ps.tile([128, 1024], f32, tag="st", name=f"pj{mt}")
                    pjs[mt] = (pf[:, 0:512], pf[:, 512:768], pf)
                else:
                    a = ps.tile([128, 512], f32, tag="mm", bufs=4,
                                name=f"pj{mt}a")
                    b = ps.tile([128, 512], f32, tag="mm", bufs=4,
                                name=f"pj{mt}b")
                    pjs[mt] = (a[:], b[:, 0:256], None)

            def pj_chain(mt, jts):
                pa, pb, _ = pjs[mt]
                for pf, o3, n3 in ((pa, 0, 512), (pb, 512, 256)):
                    for jt in jts:
                        nc.tensor.matmul(
                            pf, OUTT[:, jt, mt * 128:(mt + 1) * 128],
                            WP[:, jt, o3:o3 + n3],
                            start=(jt == 0), stop=(jt == CT - 1))

            def pj_evict(mt):
                pa, pb, _ = pjs.pop(mt)
                f = sb.tile([128, C], f32, tag="ftile")
                nc.scalar.copy(f[:, 0:512], pa)
                nc.scalar.copy(f[:, 512:768], pb)
                nc.sync.dma_start(
                    out=d_out.ap()[mt * 128:(mt + 1) * 128, :], in_=f)

            pj_alloc(0, "st")
            pj_chain(0, range(5))
            pj_alloc(1, "st")
            pj_chain(1, range(5))
            ut_mms(11, range(6, 8))
            ut_norm(11)
            pj_alloc(2, "mm")
            pj_chain(2, range(5))
            pj_alloc(3, "mm")
            pj_chain(3, range(5))
            for mt in range(4):
                pj_chain(mt, range(5, CT))
                pj_evict(mt)
            for mt in range(4, 8):
                pj_alloc(mt, "st" if mt % 2 == 0 else "mm")
                pj_chain(mt, range(CT))
                pj_evict(mt)

    with tile.TileContext(nc) as tc:
        if loop_k and loop_k > 1:
            with tc.For_i(0, loop_k, 1):
                body(tc)
        else:
            body(tc)

    nc.compile()
    return nc


def _prep(x, qkv_w, qkv_b, proj_w, proj_b, rel_pos_h, rel_pos_w):
    import ml_dtypes
    f = lambda a: np.asarray(a, dtype=np.float32)
    x, qkv_w, proj_w = f(x), f(qkv_w), f(proj_w)
    rel_pos_h, rel_pos_w = f(rel_pos_h), f(rel_pos_w)
    assert not np.any(f(qkv_b)) and not np.any(f(proj_b)), \
        "nonzero qkv/proj bias not supported by this kernel build"

    bf = ml_dtypes.bfloat16
    B = x.shape[0]

    def ctfold(a):
        # [C, N] -> [128, CT*N] with a[ct*128+p, n] at [p, ct, n]
        N = a.shape[1]
        return np.ascontiguousarray(
            a.reshape(CT, 128, N).transpose(1, 0, 2).reshape(128, CT * N))

    xT = np.ascontiguousarray(
        x.reshape(B, HW, C).transpose(0, 2, 1))           # (B, C, HW)
    xT = np.stack([ctfold(xT[b]) for b in range(B)]).astype(bf)
    wqk = ctfold(np.concatenate(
        [qkv_w[0:C] * np.float32(HD ** -0.5), qkv_w[C:2 * C]], 0).T).astype(bf)
    wv = ctfold(qkv_w[2 * C:3 * C].T).astype(bf)
    wp = ctfold(proj_w.T).astype(bf)

    idx = np.arange(32)[:, None] - np.arange(32)[None, :] + 31   # (h, k)
    sc = np.float32(HD ** 0.5)
    rha = np.ascontiguousarray(
        (rel_pos_h[idx] * sc).transpose(2, 0, 1).reshape(HD, HW)).astype(bf)
    rwa = np.ascontiguousarray(
        (rel_pos_w[idx] * sc).transpose(2, 0, 1).reshape(HD, HW)).astype(bf)

    # E patterns: rows 0..31 block indicator (n//32 == r), rows 32..63 stripe
    # indicator (n%32 == r); these turn the precomputed rel_hT/rel_wT rows of
    # the S.T rhs into the broadcast bias layout during the fused matmul.
    ep = np.zeros((HD, HW), bf)
    n = np.arange(HW)
    ep[n // 32, n] = 1.0
    ep[32 + n % 32, n] = 1.0
    ep = np.ascontiguousarray(np.tile(ep, (1, NH)))
    return xT, {"wqk": wqk, "wv": wv, "wp": wp, "rha": rha, "rwa": rwa, "ep": ep}


def _make_runner(nc):
    """Build a cached jitted 8-core runner for the compiled Bass module."""
    import jax
    import concourse.mybir as mybir
    from concourse.bass2jax import (
        _bass_exec_p, install_neuronx_cc_hook, partition_id_tensor)
    from jax.experimental.shard_map import shard_map
    from jax.sharding import Mesh, PartitionSpec

    install_neuronx_cc_hook()
    partition_name = nc.partition_id_tensor.name if nc.partition_id_tensor else None
    in_names, out_names, out_avals, zero_outs = [], [], [], []
    for alloc in nc.m.functions[0].allocations:
        if not isinstance(alloc, mybir.MemoryLocationSet):
            continue
        name = alloc.memorylocations[0].name
        if alloc.kind == "ExternalInput":
            if name != partition_name:
                in_names.append(name)
        elif alloc.kind == "ExternalOutput":
            shape = tuple(alloc.tensor_shape)
            dtype = mybir.dt.np(alloc.dtype)
            out_names.append(name)
            out_avals.append(jax.core.ShapedArray(shape, dtype))
            zero_outs.append(np.zeros(shape, dtype))
    n_params = len(in_names)
    n_outs = len(out_avals)
    all_in_names = list(in_names) + list(out_names)
    if partition_name is not None:
        all_in_names.append(partition_name)

    def _body(*args):
        operands = list(args)
        if partition_name is not None:
            operands.append(partition_id_tensor())
        return tuple(_bass_exec_p.bind(
            *operands,
            out_avals=tuple(out_avals),
            in_names=tuple(all_in_names),
            out_names=tuple(out_names),
            lowering_input_output_aliases=(),
            sim_require_finite=True,
            sim_require_nnan=True,
            nc=nc,
        ))

    devices = jax.devices()[:NCORES]
    mesh = Mesh(np.asarray(devices), ("core",))
    spec = jax.sharding.NamedSharding(mesh, PartitionSpec("core"))
    sharded = jax.jit(
        shard_map(_body, mesh=mesh,
                  in_specs=(PartitionSpec("core"),) * (n_params + n_outs),
                  out_specs=(PartitionSpec("core"),) * n_outs,
                  check_rep=False),
        keep_unused=True)

    class Runner:
        def __init__(self):
            self._dev_args = None

        def put(self, in_maps):
            concat_in = [
                np.concatenate([np.asarray(m[name]) for m in in_maps], axis=0)
                for name in in_names
            ]
            concat_zeros = [
                np.zeros((NCORES * z.shape[0], *z.shape[1:]), z.dtype)
                for z in zero_outs
            ]
            self._dev_args = [jax.device_put(a, spec)
                              for a in concat_in + concat_zeros]
            jax.block_until_ready(self._dev_args)

        def exec(self):
            out = sharded(*self._dev_args)
            jax.block_until_ready(out)
            return out

        def run(self, in_maps):
            self.put(in_maps)
            out_arrs = [np.asarray(a) for a in self.exec()]
            self._dev_args = None
            return [
                {name: out_arrs[i].reshape(NCORES, *out_avals[i].shape)[c]
                 for i, name in enumerate(out_names)}
                for c in range(NCORES)
            ]

    return Runner()


def get_runner(_loop_k=0):
    key = ("runner", _loop_k)
    if key not in _CACHE:
        nc = _build(loop_k=_loop_k)
        _CACHE[key] = _make_runner(nc)
    return _CACHE[key]


def kernel(x, qkv_w, qkv_b, proj_w, proj_b, rel_pos_h, rel_pos_w, _loop_k=0):
    xT, shared = _prep(x, qkv_w, qkv_b, proj_w, proj_b, rel_pos_h, rel_pos_w)
    B = xT.shape[0]
    assert B == NCORES
    runner = get_runner(_loop_k)
    results = runner.run([{"xT": xT[b], **shared} for b in range(B)])
    out = np.stack([results[b]["out"] for b in range(B)], 0)
    return out.reshape(B, H, W, C)
